# revision 17
# baseline (speedup 1.0000x reference)
"""AttentionBlock (GroupNorm + MHA + out-proj + residual) on 8 trn2 NeuronCores.

Data-parallel over batch: B=16 -> 2 batches per core. v2, restructured for PE
density (PE matmul work ~164us/core is the roofline):

  - Prologue: per-chunk GroupNorm (groups never span 128-channel chunks, so
    stats/normalize pipeline per chunk as its DMA lands), critical weight
    slices (qk m=0/m=4, v) DMA'd first, prologue qk GEMMs borrow the idle
    S-PSUM banks. First exp should start ~11us instead of ~43us.
  - Attention emitted as one flat (batch, head, j) stream with one-step S
    lookahead: per step we emit exp(cur), S(next), PV(cur) so the in-order
    PE queue never head-of-line blocks on an exp dependency at head/batch
    boundaries. Fillers (later qk chunks, v^T, next-batch prep, prior-batch
    out-proj) are demoted and soak PE slack.
  - Per-head drain: one [65,1024] PSUM->SBUF copy (O rows + softmax-sum row
    together), reciprocal on DVE, broadcast via DRAM bounce, multiply on
    GpSimd. Last two heads of batch 1 use a PE-broadcast fast path to cut
    the tail latency.
  - Batch-1 group stats use DVE (square+reduce) instead of ScalarE so the
    exp stream stays dense.
  - Final out-proj runs with 4 PSUM tiles in parallel (st/o/gp rings are
    free by then), bias+residual fused, per-chunk y DMA.
"""
import os
import sys

sys.path.insert(0, "/opt/trn_rl_repo")

import numpy as np

import concourse.bass as bass
import concourse.tile as tile
from concourse import bacc, mybir
from concourse import bass_utils

F32 = mybir.dt.float32
BF16 = mybir.dt.bfloat16
MMDT = BF16 if os.environ.get("K_BF16", "1") == "1" else mybir.dt.float32r
AF = mybir.ActivationFunctionType
OP = mybir.AluOpType
X = mybir.AxisListType.X

NB = 2          # batches per core
C = 512         # channels
HW = 1024       # tokens
NH = 8          # heads
NG = 8          # groups
NC_CH = 4       # channel chunks of 128
EPS = 1e-5
GSIZE = C // NG * HW  # elements per group = 65536


def build_program(nc, tc, ctx):
    x_d = nc.dram_tensor("x", [NB, C, HW], F32, kind="ExternalInput").ap()
    wt_d = nc.dram_tensor("wt", [C, 3 * C], MMDT, kind="ExternalInput").ap()
    wto_d = nc.dram_tensor("wto", [C, C], MMDT, kind="ExternalInput").ap()
    bq_d = nc.dram_tensor("bq", [128, 8], F32, kind="ExternalInput").ap()
    gam_d = nc.dram_tensor("gam", [128, NC_CH], F32, kind="ExternalInput").ap()
    bet_d = nc.dram_tensor("bet", [128, NC_CH], F32, kind="ExternalInput").ap()
    bout_d = nc.dram_tensor("bout", [128, NC_CH], F32, kind="ExternalInput").ap()
    gmask_d = nc.dram_tensor("gmask", [128, NC_CH, 2], F32, kind="ExternalInput").ap()
    gmaskT_d = nc.dram_tensor("gmaskT", [2, NC_CH, 128], F32, kind="ExternalInput").ap()
    vtones_d = nc.dram_tensor("vtones", [128, NH, NH, 1], MMDT, kind="ExternalInput").ap()
    onesb_d = nc.dram_tensor("onesb", [1, 64], MMDT, kind="ExternalInput").ap()
    y_d = nc.dram_tensor("y", [NB, C, HW], F32, kind="ExternalOutput").ap()

    consts = ctx.enter_context(tc.tile_pool(name="consts", bufs=1))
    x_pool = ctx.enter_context(tc.tile_pool(name="x", bufs=2))
    xn_pool = ctx.enter_context(tc.tile_pool(name="xn", bufs=2))
    qkc_pool = ctx.enter_context(tc.tile_pool(name="qkc", bufs=10))
    vt_pool = ctx.enter_context(tc.tile_pool(name="vt", bufs=2))
    o_pool = ctx.enter_context(tc.tile_pool(name="o", bufs=2))
    p_pool = ctx.enter_context(tc.tile_pool(name="p", bufs=16))
    ou_pool = ctx.enter_context(tc.tile_pool(name="ou", bufs=2))
    y_pool = ctx.enter_context(tc.tile_pool(name="y", bufs=2))
    gn_pool = ctx.enter_context(tc.tile_pool(name="gn", bufs=2))
    r_pool = ctx.enter_context(tc.tile_pool(name="r", bufs=3))
    rb_pool = ctx.enter_context(tc.tile_pool(name="rb", bufs=2))
    dram = ctx.enter_context(tc.tile_pool(name="dram", bufs=2, space="DRAM"))
    # PSUM: 8 banks = st(2x2) + o(2) + gp(2)
    ps_st_pool = ctx.enter_context(tc.tile_pool(name="ps_st", bufs=2, space="PSUM"))
    ps_o_pool = ctx.enter_context(tc.tile_pool(name="ps_o", bufs=1, space="PSUM"))
    ps_gp_pool = ctx.enter_context(tc.tile_pool(name="ps_gp", bufs=1, space="PSUM"))

    def demoted():
        return tc.high_priority(offset=-1000000)

    # ---- consts (small ones first; they feed warm-up + groupnorm) ----
    gmask = consts.tile([128, NC_CH, 2], F32)
    nc.sync.dma_start(gmask[:], gmask_d)
    gmaskT = consts.tile([2, NC_CH, 128], F32)
    nc.sync.dma_start(gmaskT[:], gmaskT_d)
    bq_sb = consts.tile([128, 8], F32)
    nc.sync.dma_start(bq_sb[:], bq_d)
    gam_sb = consts.tile([128, NC_CH], F32)
    nc.sync.dma_start(gam_sb[:], gam_d)
    bet_sb = consts.tile([128, NC_CH], F32)
    nc.sync.dma_start(bet_sb[:], bet_d)
    onesb = consts.tile([1, 64], MMDT)
    nc.sync.dma_start(onesb[:], onesb_d)
    bout_sb = consts.tile([128, NC_CH], F32)
    wt_sb = consts.tile([128, NC_CH, 3 * C], MMDT)
    wto_sb = consts.tile([128, NC_CH, C], MMDT)

    def load_wt_slice(lo, hi):
        nc.sync.dma_start(
            wt_sb[:, :, lo:hi],
            wt_d[:, lo:hi].rearrange("(c p) o -> p c o", p=128),
        )

    # ---- batch-0 x load: 4 chunk DMAs issued immediately ----
    x0 = x_pool.tile([128, NC_CH, HW], F32, name="x0", tag="x")
    for c in range(NC_CH):
        nc.sync.dma_start(x0[:, c, :], x_d[0, c * 128 : (c + 1) * 128, :])
    # critical weight slices next (qk m=0 / m=4, then v for vt0)
    load_wt_slice(0, 128)
    load_wt_slice(512, 640)
    load_wt_slice(2 * C, 3 * C)

    # ---- PE warm-up riding on x chunks as they land (p-state ramp).
    # Writes the ps_o slot, which has no real user until PV(0,0,0).
    wu_ps = ps_o_pool.tile([128, HW], F32, tag="po")
    for c in range(NC_CH):
        for w in range(3):
            nc.tensor.matmul(
                wu_ps[0:2, 0:512],
                gmask[:, c, :],
                x0[:, c, 0:512].bitcast(F32),
                start=True, stop=True,
            )

    xn = {}
    qk = {0: {}, 1: {}}
    vt = {}
    o_sb = {}
    x_t = {0: x0}

    def groupnorm_chunk(b, c, x_sb, xn_sb, stats2, gnps, scalar_sq):
        """Per-chunk group stats + normalize. Groups 2c,2c+1 live wholly in
        chunk c. gnps cols 0:8 hold raw (sum,sumsq) per chunk; cols 8:16 the
        broadcast (mean,rstd) per chunk."""
        nc.vector.reduce_sum(stats2[:, c, 0:1], x_sb[:, c, :], axis=X)
        if scalar_sq:
            scr = p_pool.tile([128, HW], MMDT, tag="p_t", name=f"sq{b}_{c}")
            nc.scalar.activation(
                scr[:], x_sb[:, c, :], AF.Square,
                accum_out=stats2[:, c, 1:2],
            )
        else:
            scr = p_pool.tile([128, HW], MMDT, tag="p_t", name=f"sq{b}_{c}")
            nc.vector.tensor_tensor(scr[:], x_sb[:, c, :], x_sb[:, c, :], op=OP.mult)
            nc.vector.reduce_sum(stats2[:, c, 1:2], scr[:], axis=X)
        nc.tensor.matmul(
            gnps[0:2, 2 * c : 2 * c + 2],
            gmask[:, c, :],
            stats2[:, c, :],
            start=True, stop=True,
        )
        msr = gn_pool.tile([2, 2], F32, tag="msr", name=f"msr{b}_{c}")
        var = gn_pool.tile([2, 1], F32, tag="var", name=f"var{b}_{c}")
        nc.vector.tensor_scalar_mul(msr[:], gnps[0:2, 2 * c : 2 * c + 2], 1.0 / GSIZE)
        nc.vector.tensor_tensor(var[:], msr[:, 0:1], msr[:, 0:1], op=OP.mult)
        nc.vector.tensor_tensor(var[:], msr[:, 1:2], var[:], op=OP.subtract)
        nc.vector.tensor_scalar_add(var[:], var[:], EPS)
        nc.vector.reciprocal(var[:], var[:])
        nc.scalar.activation(msr[:, 1:2], var[:], AF.Sqrt)  # rstd
        nc.tensor.matmul(
            gnps[:, 8 + 2 * c : 8 + 2 * c + 2],
            gmaskT[:, c, :],
            msr[:],
            start=True, stop=True,
        )
        ab = gn_pool.tile([128, 2], F32, tag="ab", name=f"ab{b}_{c}")
        tmp = gn_pool.tile([128, 1], F32, tag="tmp", name=f"tmp{b}_{c}")
        nc.vector.tensor_tensor(
            ab[:, 0:1], gnps[:, 8 + 2 * c + 1 : 8 + 2 * c + 2],
            gam_sb[:, c : c + 1], op=OP.mult,
        )
        nc.vector.tensor_tensor(
            tmp[:], gnps[:, 8 + 2 * c : 8 + 2 * c + 1], ab[:, 0:1], op=OP.mult
        )
        nc.vector.tensor_tensor(
            ab[:, 1:2], bet_sb[:, c : c + 1], tmp[:], op=OP.subtract
        )
        nc.vector.tensor_scalar(
            out=xn_sb[:, c, :],
            in0=x_sb[:, c, :],
            scalar1=ab[:, 0:1],
            scalar2=ab[:, 1:2],
            op0=OP.mult,
            op1=OP.add,
        )

    def qk_chunk(b, m, ps_pool, tag):
        """q/k output channels m*128..(m+1)*128 in [ch, tok] layout."""
        ps_qk = ps_pool.tile([128, HW], F32, tag=tag, name=f"psqk{b}_{m}")
        for c in range(NC_CH):
            for half in range(2):
                nc.tensor.matmul(
                    ps_qk[:, half * 512 : (half + 1) * 512],
                    wt_sb[:, c, m * 128 : (m + 1) * 128],
                    xn[b][:, c, half * 512 : (half + 1) * 512],
                    start=(c == 0),
                    stop=(c == NC_CH - 1),
                )
        qk_t = qkc_pool.tile([128, HW], MMDT, tag="qkc", name=f"qk{b}_{m}")
        nc.vector.tensor_scalar_add(qk_t[:], ps_qk[:], bq_sb[:, m : m + 1])
        qk[b][m] = qk_t

    def vt_group(b, jp):
        """v^T tile rows for token chunks j=2jp,2jp+1 (with ones column)."""
        vt_sb = vt[b]
        vt_v = vt_sb.rearrange("p j (h e) -> p j h e", e=65)
        ps_vt = ps_gp_pool.tile([128, HW], F32, tag="gp", name=f"psvt{b}_{jp}")
        for j2 in range(2):
            j = 2 * jp + j2
            for c in range(NC_CH):
                nc.tensor.matmul(
                    ps_vt[:, j2 * 512 : (j2 + 1) * 512],
                    xn[b][:, c, j * 128 : (j + 1) * 128],
                    wt_sb[:, c, 2 * C : 3 * C],
                    start=(c == 0),
                    stop=(c == NC_CH - 1),
                )
        nc.vector.tensor_copy(
            vt_v[:, 2 * jp : 2 * jp + 2, :, 0:64],
            ps_vt[:].rearrange("p (j h e) -> p j h e", j=2, h=NH),
        )

    def new_vt(b):
        vt_sb = vt_pool.tile([128, NH, NH * 65], MMDT, tag="vt", name=f"vt{b}")
        vt_v = vt_sb.rearrange("p j (h e) -> p j h e", e=65)
        nc.sync.dma_start(vt_v[:, :, :, 64:65], vtones_d)
        vt[b] = vt_sb

    def outproj_r(b, r, ps_pool, tag, final=False):
        probe = os.environ.get("K_PROBE", "")
        if probe:
            y_t = y_pool.tile([128, HW], F32, tag="y", name=f"y{b}_{r}")
            src = {"xn": lambda: xn[b][:, r, :],
                   "o": lambda: o_sb[b][:, r, :],
                   "qk": lambda: qk[b][r][:, :]}[probe]()
            nc.vector.tensor_copy(y_t[:], src)
            nc.sync.dma_start(y_d[b, r * 128 : (r + 1) * 128, :], y_t[:])
            return
        ps_y = ps_pool.tile([128, HW], F32, tag=tag, name=f"psy{b}_{r}")
        for c in range(NC_CH):
            for half in range(2):
                nc.tensor.matmul(
                    ps_y[:, half * 512 : (half + 1) * 512],
                    wto_sb[:, c, r * 128 : (r + 1) * 128],
                    o_sb[b][:, c, half * 512 : (half + 1) * 512],
                    start=(c == 0),
                    stop=(c == NC_CH - 1),
                )
        y_t = y_pool.tile([128, HW], F32, tag="y", name=f"y{b}_{r}")
        nc.vector.scalar_tensor_tensor(
            out=y_t[:],
            in0=ps_y[:],
            scalar=bout_sb[:, r : r + 1],
            in1=x_t[b][:, r, :],
            op0=OP.add,
            op1=OP.add,
        )
        nc.sync.dma_start(y_d[b, r * 128 : (r + 1) * 128, :], y_t[:])

    # ---- batch-0 prologue: per-chunk groupnorm, qk m=0/m=4 interleaved ----
    xn[0] = xn_pool.tile([128, NC_CH, HW], MMDT, name="xn0", tag="xn")
    stats2_0 = gn_pool.tile([128, NC_CH, 2], F32, tag="stats", name="stats0")
    gnps0 = ps_gp_pool.tile([128, 16], F32, tag="gp", name="gnps0")
    ps_qk0 = ps_st_pool.tile([128, HW], F32, tag="st", name="psqk0_0")
    ps_qk4 = ps_st_pool.tile([128, HW], F32, tag="st", name="psqk0_4")
    for c in range(NC_CH):
        groupnorm_chunk(0, c, x0, xn[0], stats2_0, gnps0, scalar_sq=True)
        for ps_t, m in ((ps_qk0, 0), (ps_qk4, 4)):
            for half in range(2):
                nc.tensor.matmul(
                    ps_t[:, half * 512 : (half + 1) * 512],
                    wt_sb[:, c, m * 128 : (m + 1) * 128],
                    xn[0][:, c, half * 512 : (half + 1) * 512],
                    start=(c == 0),
                    stop=(c == NC_CH - 1),
                )
    qk_t0 = qkc_pool.tile([128, HW], MMDT, tag="qkc", name="qk0_0")
    nc.vector.tensor_scalar_add(qk_t0[:], ps_qk0[:], bq_sb[:, 0:1])
    qk[0][0] = qk_t0
    qk_t4 = qkc_pool.tile([128, HW], MMDT, tag="qkc", name="qk0_4")
    nc.vector.tensor_scalar_add(qk_t4[:], ps_qk4[:], bq_sb[:, 4:5])
    qk[0][4] = qk_t4
    new_vt(0)
    o_sb[0] = o_pool.tile([128, NC_CH, HW], MMDT, name="o0", tag="o")

    # ---- attention: flat (b, h, j) stream with one-step S lookahead ----
    steps = [(b, h, j) for b in range(NB) for h in range(NH) for j in range(8)]

    def s_mm(b, h, j):
        st = ps_st_pool.tile([128, HW], F32, tag="st", name=f"st{b}_{h}_{j}")
        po = 64 * (h % 2)
        q_ap = qk[b][h // 2][po : po + 64, :]
        k_ap = qk[b][4 + h // 2][po : po + 64, :]
        for half in range(2):
            nc.tensor.matmul(
                st[:, half * 512 : (half + 1) * 512],
                k_ap[:, j * 128 : (j + 1) * 128],
                q_ap[:, half * 512 : (half + 1) * 512],
                start=True,
                stop=True,
            )
        return st

    def fillers(b, h, j):
        """Demoted background work keyed to step positions."""
        if b == 0:
            if h == 0 and j == 0:
                load_wt_slice(128, 256)      # wt m=1
                vt_group(0, 0)
            elif h == 0 and j == 2:
                load_wt_slice(640, 768)      # wt m=5
                vt_group(0, 1)
            elif h == 0 and j == 4:
                load_wt_slice(256, 384)      # wt m=2
                vt_group(0, 2)
            elif h == 0 and j == 6:
                load_wt_slice(768, 896)      # wt m=6
                vt_group(0, 3)
            elif h == 1 and j == 0:
                load_wt_slice(384, 512)      # wt m=3
                load_wt_slice(896, 1024)     # wt m=7
                x1 = x_pool.tile([128, NC_CH, HW], F32, name="x1", tag="x")
                for c in range(NC_CH):
                    nc.sync.dma_start(x1[:, c, :], x_d[1, c * 128 : (c + 1) * 128, :])
                x_t[1] = x1
                qk_chunk(0, 1, ps_gp_pool, "gp")
            elif h == 1 and j == 4:
                nc.sync.dma_start(
                    wto_sb[:], wto_d.rearrange("(c p) o -> p c o", p=128)
                )
                nc.sync.dma_start(bout_sb[:], bout_d)
                qk_chunk(0, 5, ps_gp_pool, "gp")
            elif h == 2 and j == 0:
                qk_chunk(0, 2, ps_gp_pool, "gp")
            elif h == 2 and j == 4:
                qk_chunk(0, 6, ps_gp_pool, "gp")
            elif h == 3 and j == 0:
                xn[1] = xn_pool.tile([128, NC_CH, HW], MMDT, name="xn1", tag="xn")
                stats2_1 = gn_pool.tile(
                    [128, NC_CH, 2], F32, tag="stats", name="stats1"
                )
                gnps1 = ps_gp_pool.tile([128, 16], F32, tag="gp", name="gnps1")
                for c in range(NC_CH):
                    groupnorm_chunk(1, c, x_t[1], xn[1], stats2_1, gnps1,
                                    scalar_sq=False)
            elif h == 4 and j == 0:
                qk_chunk(0, 3, ps_gp_pool, "gp")
            elif h == 4 and j == 4:
                qk_chunk(0, 7, ps_gp_pool, "gp")
            elif h == 5 and j == 0:
                qk_chunk(1, 0, ps_gp_pool, "gp")
            elif h == 5 and j == 4:
                qk_chunk(1, 4, ps_gp_pool, "gp")
            elif h == 6 and j == 0:
                new_vt(1)
                vt_group(1, 0)
            elif h == 7 and j == 0:
                vt_group(1, 1)
            elif h == 7 and j == 4:
                vt_group(1, 2)
        else:
            if h == 0 and j == 0:
                vt_group(1, 3)
            elif h == 0 and j == 4:
                qk_chunk(1, 1, ps_gp_pool, "gp")
            elif h == 1 and j == 0:
                qk_chunk(1, 5, ps_gp_pool, "gp")
            elif h == 1 and j == 4:
                outproj_r(0, 0, ps_gp_pool, "gp")
            elif h == 2 and j == 0:
                qk_chunk(1, 2, ps_gp_pool, "gp")
            elif h == 2 and j == 4:
                outproj_r(0, 1, ps_gp_pool, "gp")
            elif h == 3 and j == 0:
                qk_chunk(1, 6, ps_gp_pool, "gp")
            elif h == 3 and j == 4:
                outproj_r(0, 2, ps_gp_pool, "gp")
            elif h == 4 and j == 0:
                qk_chunk(1, 3, ps_gp_pool, "gp")
            elif h == 4 and j == 4:
                outproj_r(0, 3, ps_gp_pool, "gp")
            elif h == 5 and j == 0:
                qk_chunk(1, 7, ps_gp_pool, "gp")

    def norm_head(b, h, ps_o, fast):
        """Drain PSUM (O rows + sum row in one copy), normalize into o_sb."""
        if os.environ.get("K_BASENORM", "0") == "1":
            norm_head_base(b, h, ps_o)
            return
        po = 64 * (h % 2)
        o65 = ou_pool.tile([65, HW], F32, tag="ou", name=f"o65_{b}_{h}")
        nc.vector.tensor_copy(o65[:], ps_o[0:65, :])
        # reciprocal must stay on the sum row's own partition (the custom
        # DVE op does not support a cross-partition shift)
        r65 = r_pool.tile([65, HW], F32, tag="row", name=f"r{b}_{h}")
        if os.environ.get("K_RECIP", "std") == "fast":
            nc.vector.reciprocal_approx_fast(r65[64:65, :], o65[64:65, :])
        else:
            nc.vector.reciprocal(r65[64:65, :], o65[64:65, :])
        if fast:
            r1b = r_pool.tile([1, HW], MMDT, tag="rowb", name=f"rb16_{b}_{h}")
            nc.vector.tensor_copy(r1b[:], r65[64:65, :])
            bc_ps = ps_gp_pool.tile([64, HW], F32, tag="gp", name=f"bc{b}_{h}")
            for half in range(2):
                nc.tensor.matmul(
                    bc_ps[:, half * 512 : (half + 1) * 512],
                    onesb[:],
                    r1b[:, half * 512 : (half + 1) * 512],
                    start=True,
                    stop=True,
                )
            nc.vector.tensor_tensor(
                o_sb[b][po : po + 64, h // 2, :], o65[0:64, :], bc_ps[0:64, :],
                op=OP.mult,
            )
        else:
            dr = dram.tile([1, HW], F32, name=f"dr{b}_{h}")
            nc.sync.dma_start(dr[:], r65[64:65, :])
            rb = rb_pool.tile([64, HW], F32, tag="rb", name=f"rbb{b}_{h}")
            nc.sync.dma_start(rb[:], dr[:].to_broadcast((64, HW)))
            nc.gpsimd.tensor_tensor(
                o_sb[b][po : po + 64, h // 2, :], o65[0:64, :], rb[:], op=OP.mult
            )

    def norm_head_base(b, h, ps_o):
        """Baseline-style drain + DRAM-bounce normalization."""
        po = 64 * (h % 2)
        o_un = ou_pool.tile([64, HW], F32, tag="ou", name=f"oun_{b}_{h}")
        nc.vector.tensor_copy(o_un[:], ps_o[0:64, :])
        s_row = r_pool.tile([1, HW], F32, tag="row", name=f"s{b}_{h}")
        nc.vector.tensor_copy(s_row[:], ps_o[64:65, :])
        r_row = r_pool.tile([1, HW], F32, tag="row", name=f"r{b}_{h}")
        nc.vector.reciprocal_approx_fast(r_row[:], s_row[:])
        dr = dram.tile([1, HW], F32, name=f"dr{b}_{h}")
        nc.sync.dma_start(dr[:], r_row[:])
        rb = rb_pool.tile([64, HW], F32, tag="rb", name=f"rbb{b}_{h}")
        nc.sync.dma_start(rb[:], dr[:].to_broadcast((64, HW)))
        eng = nc.gpsimd if not (b == 1 and h == 7) else nc.vector
        eng.tensor_tensor(
            o_sb[b][po : po + 64, h // 2, :], o_un[:], rb[:], op=OP.mult
        )

    pend_st = s_mm(0, 0, 0)
    ps_o = None
    for idx, (b, h, j) in enumerate(steps):
        st = pend_st
        p_t = p_pool.tile([128, HW], MMDT, tag="p_t", name=f"p{b}_{h}_{j}")
        nc.scalar.activation(p_t[:], st[:], AF.Exp, scale=0.125)
        with demoted():
            fillers(b, h, j)
        if idx + 1 < len(steps):
            nb_, nh_, nj_ = steps[idx + 1]
            pend_st = s_mm(nb_, nh_, nj_)
        if j == 0:
            if b == 1 and h == 0:
                o_sb[1] = o_pool.tile([128, NC_CH, HW], MMDT, name="o1", tag="o")
            ps_o = ps_o_pool.tile([128, HW], F32, tag="po", name=f"po{b}_{h}")
        for half in range(2):
            nc.tensor.matmul(
                ps_o[0:65, half * 512 : (half + 1) * 512],
                vt[b][:, j, 65 * h : 65 * h + 65],
                p_t[:, half * 512 : (half + 1) * 512],
                start=(j == 0),
                stop=(j == 7),
            )
        if j == 7:
            fast = b == 1 and h >= 6 and os.environ.get("K_FAST", "1") == "1"
            dem = (b == 0 and os.environ.get("K_DEMOTE", "1") == "1"
                   and os.environ.get("K_BASENORM", "0") != "1")
            if dem:
                with demoted():
                    norm_head(b, h, ps_o, fast)
            else:
                norm_head(b, h, ps_o, fast)

    # ---- final out-proj for batch 1: 4 PSUM tiles in parallel ----
    outproj_r(1, 0, ps_st_pool, "st", final=True)
    outproj_r(1, 1, ps_st_pool, "st", final=True)
    outproj_r(1, 2, ps_o_pool, "po", final=True)
    outproj_r(1, 3, ps_gp_pool, "gp", final=True)


_NC_CACHE = None


def _build():
    global _NC_CACHE
    if _NC_CACHE is not None:
        return _NC_CACHE
    import contextlib

    nc = bacc.Bacc("TRN2", target_bir_lowering=False, debug=False)
    with tile.TileContext(nc) as tc:
        with contextlib.ExitStack() as ctx:
            build_program(nc, tc, ctx)
    nc.compile()
    _NC_CACHE = nc
    return nc


def make_in_maps(x, gamma, beta, w_qkv, b_qkv, w_out, b_out):
    x = np.ascontiguousarray(np.asarray(x, dtype=np.float32))
    gamma = np.asarray(gamma, dtype=np.float32)
    beta = np.asarray(beta, dtype=np.float32)
    w_qkv = np.asarray(w_qkv, dtype=np.float32)
    b_qkv = np.asarray(b_qkv, dtype=np.float32)
    w_out = np.asarray(w_out, dtype=np.float32)
    b_out = np.asarray(b_out, dtype=np.float32)

    B, Cc, H, W = x.shape
    assert (B, Cc, H, W) == (16, 512, 32, 32)

    # host-side weight layout transforms (pure layout; no compute moved
    # off-device except the exact fold of the v-bias: softmax rows sum to 1,
    # so attn @ (v + b_v 1^T) = attn @ v + b_v, and W_out @ b_v folds into b_out)
    wt = np.ascontiguousarray(w_qkv.T)                      # [512, 1536]
    wto = np.ascontiguousarray(w_out.T)                     # [512, 512]
    if os.environ.get("K_BF16", "1") == "1":
        import ml_dtypes

        mmnp = ml_dtypes.bfloat16
    else:
        mmnp = np.float32
    wt = wt.astype(mmnp)
    wto = wto.astype(mmnp)
    b_out_eff = b_out + w_out @ b_qkv[2 * C : 3 * C]
    bq = np.ascontiguousarray(b_qkv[: 2 * C].reshape(8, 128).T)   # [128, 8]
    gam = np.ascontiguousarray(gamma.reshape(NC_CH, 128).T)       # [128, 4]
    bet = np.ascontiguousarray(beta.reshape(NC_CH, 128).T)
    bout = np.ascontiguousarray(b_out_eff.reshape(NC_CH, 128).T)

    # per-chunk group masks: chunk c's 128 channels = groups 2c, 2c+1
    gmask_np = np.zeros((128, NC_CH, 2), dtype=np.float32)
    gmaskT_np = np.zeros((2, NC_CH, 128), dtype=np.float32)
    for c in range(NC_CH):
        gmask_np[0:64, c, 0] = 1.0
        gmask_np[64:128, c, 1] = 1.0
        gmaskT_np[0, c, 0:64] = 1.0
        gmaskT_np[1, c, 64:128] = 1.0

    xr = x.reshape(16, 512, 1024)
    in_maps = []
    for core in range(8):
        in_maps.append(
            {
                "x": np.ascontiguousarray(xr[2 * core : 2 * core + 2]),
                "wt": wt,
                "wto": wto,
                "bq": bq,
                "gam": gam,
                "bet": bet,
                "bout": bout,
                "gmask": gmask_np,
                "gmaskT": gmaskT_np,
                "vtones": np.ones((128, NH, NH, 1), dtype=mmnp),
                "onesb": np.ones((1, 64), dtype=mmnp),
            }
        )
    return in_maps


def kernel(x, gamma, beta, w_qkv, b_qkv, w_out, b_out):
    in_maps = make_in_maps(x, gamma, beta, w_qkv, b_qkv, w_out, b_out)
    nc = _build()
    res = bass_utils.run_bass_kernel_spmd(nc, in_maps, core_ids=list(range(8)))
    out = np.concatenate([r["y"] for r in res.results], axis=0)
    return out.reshape(16, 512, 32, 32).astype(np.float32)


# revision 21
# speedup vs baseline: 1.2740x; 1.2740x over previous
"""AttentionBlock (GroupNorm + MHA + out-proj + residual) on 8 trn2 NeuronCores.

Data-parallel over batch: B=16 -> 2 batches per core. v2, restructured for PE
density (PE matmul work ~164us/core is the roofline):

  - Prologue: per-chunk GroupNorm (groups never span 128-channel chunks, so
    stats/normalize pipeline per chunk as its DMA lands), critical weight
    slices (qk m=0/m=4, v) DMA'd first, prologue qk GEMMs borrow the idle
    S-PSUM banks. First exp should start ~11us instead of ~43us.
  - Attention emitted as one flat (batch, head, j) stream with one-step S
    lookahead: per step we emit exp(cur), S(next), PV(cur) so the in-order
    PE queue never head-of-line blocks on an exp dependency at head/batch
    boundaries. Fillers (later qk chunks, v^T, next-batch prep, prior-batch
    out-proj) are demoted and soak PE slack.
  - Per-head drain: one [65,1024] PSUM->SBUF copy (O rows + softmax-sum row
    together), reciprocal on DVE, broadcast via DRAM bounce, multiply on
    GpSimd. Last two heads of batch 1 use a PE-broadcast fast path to cut
    the tail latency.
  - Batch-1 group stats use DVE (square+reduce) instead of ScalarE so the
    exp stream stays dense.
  - Final out-proj runs with 4 PSUM tiles in parallel (st/o/gp rings are
    free by then), bias+residual fused, per-chunk y DMA.
"""
import os
import sys

sys.path.insert(0, "/opt/trn_rl_repo")

import numpy as np

import concourse.bass as bass
import concourse.tile as tile
from concourse import bacc, mybir
from concourse import bass_utils

F32 = mybir.dt.float32
BF16 = mybir.dt.bfloat16
MMDT = BF16 if os.environ.get("K_BF16", "1") == "1" else mybir.dt.float32r
AF = mybir.ActivationFunctionType
OP = mybir.AluOpType
X = mybir.AxisListType.X

NB = 2          # batches per core
C = 512         # channels
HW = 1024       # tokens
NH = 8          # heads
NG = 8          # groups
NC_CH = 4       # channel chunks of 128
EPS = 1e-5
GSIZE = C // NG * HW  # elements per group = 65536


def build_program(nc, tc, ctx):
    x_d = nc.dram_tensor("x", [NB, C, HW], F32, kind="ExternalInput").ap()
    wt_d = nc.dram_tensor("wt", [C, 3 * C], MMDT, kind="ExternalInput").ap()
    wto_d = nc.dram_tensor("wto", [C, C], MMDT, kind="ExternalInput").ap()
    bq_d = nc.dram_tensor("bq", [128, 8], F32, kind="ExternalInput").ap()
    gam_d = nc.dram_tensor("gam", [128, NC_CH], F32, kind="ExternalInput").ap()
    bet_d = nc.dram_tensor("bet", [128, NC_CH], F32, kind="ExternalInput").ap()
    bout_d = nc.dram_tensor("bout", [128, NC_CH], F32, kind="ExternalInput").ap()
    gmask_d = nc.dram_tensor("gmask", [128, NC_CH, 2], F32, kind="ExternalInput").ap()
    gmaskT_d = nc.dram_tensor("gmaskT", [2, NC_CH, 128], F32, kind="ExternalInput").ap()
    vtones_d = nc.dram_tensor("vtones", [128, NH, NH, 1], MMDT, kind="ExternalInput").ap()
    onesb_d = nc.dram_tensor("onesb", [1, 64], MMDT, kind="ExternalInput").ap()
    y_d = nc.dram_tensor("y", [NB, C, HW], F32, kind="ExternalOutput").ap()

    consts = ctx.enter_context(tc.tile_pool(name="consts", bufs=1))
    x_pool = ctx.enter_context(tc.tile_pool(name="x", bufs=2))
    xn_pool = ctx.enter_context(tc.tile_pool(name="xn", bufs=2))
    qkc_pool = ctx.enter_context(tc.tile_pool(name="qkc", bufs=10))
    vt_pool = ctx.enter_context(tc.tile_pool(name="vt", bufs=2))
    o_pool = ctx.enter_context(tc.tile_pool(name="o", bufs=2))
    p_pool = ctx.enter_context(tc.tile_pool(name="p", bufs=16))
    ou_pool = ctx.enter_context(tc.tile_pool(name="ou", bufs=2))
    y_pool = ctx.enter_context(tc.tile_pool(name="y", bufs=2))
    gn_pool = ctx.enter_context(tc.tile_pool(name="gn", bufs=2))
    r_pool = ctx.enter_context(tc.tile_pool(name="r", bufs=3))
    rb_pool = ctx.enter_context(tc.tile_pool(name="rb", bufs=2))
    dram = ctx.enter_context(tc.tile_pool(name="dram", bufs=2, space="DRAM"))
    # PSUM: 8 banks = st(2x2) + o(2) + gp(2)
    ps_st_pool = ctx.enter_context(tc.tile_pool(name="ps_st", bufs=2, space="PSUM"))
    ps_o_pool = ctx.enter_context(tc.tile_pool(name="ps_o", bufs=1, space="PSUM"))
    ps_gp_pool = ctx.enter_context(tc.tile_pool(name="ps_gp", bufs=1, space="PSUM"))

    def demoted():
        return tc.high_priority(offset=-1000000)

    # ---- batch-0 x load first: its DMA triggers gate everything ----
    x0 = x_pool.tile([128, NC_CH, HW], F32, name="x0", tag="x")
    for c in range(NC_CH):
        nc.sync.dma_start(x0[:, c, :], x_d[0, c * 128 : (c + 1) * 128, :])
    gmask = consts.tile([128, NC_CH, 2], F32)
    nc.sync.dma_start(gmask[:], gmask_d)
    gmaskT = consts.tile([2, NC_CH, 128], F32)
    nc.sync.dma_start(gmaskT[:], gmaskT_d)
    bq_sb = consts.tile([128, 8], F32)
    nc.sync.dma_start(bq_sb[:], bq_d)
    gam_sb = consts.tile([128, NC_CH], F32)
    nc.sync.dma_start(gam_sb[:], gam_d)
    bet_sb = consts.tile([128, NC_CH], F32)
    nc.sync.dma_start(bet_sb[:], bet_d)
    bout_sb = consts.tile([128, NC_CH], F32)
    wt_sb = consts.tile([128, NC_CH, 3 * C], MMDT)
    wto_sb = consts.tile([128, NC_CH, C], MMDT)

    def load_wt_slice(lo, hi):
        nc.sync.dma_start(
            wt_sb[:, :, lo:hi],
            wt_d[:, lo:hi].rearrange("(c p) o -> p c o", p=128),
        )

    # critical weight slices next (qk m=0 / m=4, then v for vt0)
    load_wt_slice(0, 128)
    load_wt_slice(512, 640)
    load_wt_slice(2 * C, 3 * C)

    # ---- PE warm-up riding on x chunks as they land (p-state ramp).
    # Writes the ps_o slot, which has no real user until PV(0,0,0).
    wu_ps = ps_o_pool.tile([128, HW], F32, tag="po")
    for c in range(NC_CH):
        for w in range(3):
            nc.tensor.matmul(
                wu_ps[0:2, 0:512],
                gmask[:, c, :],
                x0[:, c, 0:512].bitcast(F32),
                start=True, stop=True,
            )

    xn = {}
    qk = {0: {}, 1: {}}
    vt = {}
    o_sb = {}
    x_t = {0: x0}

    def groupnorm_chunk(b, c, x_sb, xn_sb, stats2, gnps, scalar_sq):
        """Per-chunk group stats + normalize. Groups 2c,2c+1 live wholly in
        chunk c. gnps cols 0:8 hold raw (sum,sumsq) per chunk; cols 8:16 the
        broadcast (mean,rstd) per chunk."""
        nc.vector.reduce_sum(stats2[:, c, 0:1], x_sb[:, c, :], axis=X)
        if scalar_sq:
            scr = p_pool.tile([128, HW], MMDT, tag="p_t", name=f"sq{b}_{c}")
            nc.scalar.activation(
                scr[:], x_sb[:, c, :], AF.Square,
                accum_out=stats2[:, c, 1:2],
            )
        else:
            scr = p_pool.tile([128, HW], MMDT, tag="p_t", name=f"sq{b}_{c}")
            nc.vector.tensor_tensor(scr[:], x_sb[:, c, :], x_sb[:, c, :], op=OP.mult)
            nc.vector.reduce_sum(stats2[:, c, 1:2], scr[:], axis=X)
        nc.tensor.matmul(
            gnps[0:2, 2 * c : 2 * c + 2],
            gmask[:, c, :],
            stats2[:, c, :],
            start=True, stop=True,
        )
        msr = gn_pool.tile([2, 2], F32, tag="msr", name=f"msr{b}_{c}")
        var = gn_pool.tile([2, 1], F32, tag="var", name=f"var{b}_{c}")
        nc.vector.tensor_scalar_mul(msr[:], gnps[0:2, 2 * c : 2 * c + 2], 1.0 / GSIZE)
        nc.vector.tensor_tensor(var[:], msr[:, 0:1], msr[:, 0:1], op=OP.mult)
        nc.vector.tensor_tensor(var[:], msr[:, 1:2], var[:], op=OP.subtract)
        nc.vector.tensor_scalar_add(var[:], var[:], EPS)
        nc.vector.reciprocal(var[:], var[:])
        nc.scalar.activation(msr[:, 1:2], var[:], AF.Sqrt)  # rstd
        nc.tensor.matmul(
            gnps[:, 8 + 2 * c : 8 + 2 * c + 2],
            gmaskT[:, c, :],
            msr[:],
            start=True, stop=True,
        )
        ab = gn_pool.tile([128, 2], F32, tag="ab", name=f"ab{b}_{c}")
        tmp = gn_pool.tile([128, 1], F32, tag="tmp", name=f"tmp{b}_{c}")
        nc.vector.tensor_tensor(
            ab[:, 0:1], gnps[:, 8 + 2 * c + 1 : 8 + 2 * c + 2],
            gam_sb[:, c : c + 1], op=OP.mult,
        )
        nc.vector.tensor_tensor(
            tmp[:], gnps[:, 8 + 2 * c : 8 + 2 * c + 1], ab[:, 0:1], op=OP.mult
        )
        nc.vector.tensor_tensor(
            ab[:, 1:2], bet_sb[:, c : c + 1], tmp[:], op=OP.subtract
        )
        nc.vector.tensor_scalar(
            out=xn_sb[:, c, :],
            in0=x_sb[:, c, :],
            scalar1=ab[:, 0:1],
            scalar2=ab[:, 1:2],
            op0=OP.mult,
            op1=OP.add,
        )

    def qk_chunk(b, m, ps_pool, tag):
        """q/k output channels m*128..(m+1)*128 in [ch, tok] layout."""
        ps_qk = ps_pool.tile([128, HW], F32, tag=tag, name=f"psqk{b}_{m}")
        for c in range(NC_CH):
            for half in range(2):
                nc.tensor.matmul(
                    ps_qk[:, half * 512 : (half + 1) * 512],
                    wt_sb[:, c, m * 128 : (m + 1) * 128],
                    xn[b][:, c, half * 512 : (half + 1) * 512],
                    start=(c == 0),
                    stop=(c == NC_CH - 1),
                )
        qk_t = qkc_pool.tile([128, HW], MMDT, tag="qkc", name=f"qk{b}_{m}")
        nc.vector.tensor_scalar_add(qk_t[:], ps_qk[:], bq_sb[:, m : m + 1])
        qk[b][m] = qk_t

    def vt_group(b, jp):
        """v^T tile rows for token chunks j=2jp,2jp+1 (with ones column)."""
        vt_sb = vt[b]
        vt_v = vt_sb.rearrange("p j (h e) -> p j h e", e=65)
        ps_vt = ps_gp_pool.tile([128, HW], F32, tag="gp", name=f"psvt{b}_{jp}")
        for j2 in range(2):
            j = 2 * jp + j2
            for c in range(NC_CH):
                nc.tensor.matmul(
                    ps_vt[:, j2 * 512 : (j2 + 1) * 512],
                    xn[b][:, c, j * 128 : (j + 1) * 128],
                    wt_sb[:, c, 2 * C : 3 * C],
                    start=(c == 0),
                    stop=(c == NC_CH - 1),
                )
        nc.vector.tensor_copy(
            vt_v[:, 2 * jp : 2 * jp + 2, :, 0:64],
            ps_vt[:].rearrange("p (j h e) -> p j h e", j=2, h=NH),
        )

    def new_vt(b):
        vt_sb = vt_pool.tile([128, NH, NH * 65], MMDT, tag="vt", name=f"vt{b}")
        vt_v = vt_sb.rearrange("p j (h e) -> p j h e", e=65)
        nc.sync.dma_start(vt_v[:, :, :, 64:65], vtones_d)
        vt[b] = vt_sb

    def outproj_r(b, r, ps_pool, tag, final=False):
        probe = os.environ.get("K_PROBE", "")
        if probe:
            y_t = y_pool.tile([128, HW], F32, tag="y", name=f"y{b}_{r}")
            src = {"xn": lambda: xn[b][:, r, :],
                   "o": lambda: o_sb[b][:, r, :],
                   "qk": lambda: qk[b][r][:, :]}[probe]()
            nc.vector.tensor_copy(y_t[:], src)
            nc.sync.dma_start(y_d[b, r * 128 : (r + 1) * 128, :], y_t[:])
            return
        ps_y = ps_pool.tile([128, HW], F32, tag=tag, name=f"psy{b}_{r}")
        for c in range(NC_CH):
            for half in range(2):
                nc.tensor.matmul(
                    ps_y[:, half * 512 : (half + 1) * 512],
                    wto_sb[:, c, r * 128 : (r + 1) * 128],
                    o_sb[b][:, c, half * 512 : (half + 1) * 512],
                    start=(c == 0),
                    stop=(c == NC_CH - 1),
                )
        y_t = y_pool.tile([128, HW], F32, tag="y", name=f"y{b}_{r}")
        nc.vector.scalar_tensor_tensor(
            out=y_t[:],
            in0=ps_y[:],
            scalar=bout_sb[:, r : r + 1],
            in1=x_t[b][:, r, :],
            op0=OP.add,
            op1=OP.add,
        )
        nc.sync.dma_start(y_d[b, r * 128 : (r + 1) * 128, :], y_t[:])

    # ---- batch-0 prologue: per-chunk groupnorm, qk m=0/m=4 interleaved ----
    xn[0] = xn_pool.tile([128, NC_CH, HW], MMDT, name="xn0", tag="xn")
    stats2_0 = gn_pool.tile([128, NC_CH, 2], F32, tag="stats", name="stats0")
    gnps0 = ps_gp_pool.tile([128, 16], F32, tag="gp", name="gnps0")
    ps_qk0 = ps_st_pool.tile([128, HW], F32, tag="st", name="psqk0_0")
    ps_qk4 = ps_st_pool.tile([128, HW], F32, tag="st", name="psqk0_4")
    for c in range(NC_CH):
        groupnorm_chunk(0, c, x0, xn[0], stats2_0, gnps0, scalar_sq=True)
        for ps_t, m in ((ps_qk0, 0), (ps_qk4, 4)):
            for half in range(2):
                nc.tensor.matmul(
                    ps_t[:, half * 512 : (half + 1) * 512],
                    wt_sb[:, c, m * 128 : (m + 1) * 128],
                    xn[0][:, c, half * 512 : (half + 1) * 512],
                    start=(c == 0),
                    stop=(c == NC_CH - 1),
                )
    qk_t0 = qkc_pool.tile([128, HW], MMDT, tag="qkc", name="qk0_0")
    nc.vector.tensor_scalar_add(qk_t0[:], ps_qk0[:], bq_sb[:, 0:1])
    qk[0][0] = qk_t0
    qk_t4 = qkc_pool.tile([128, HW], MMDT, tag="qkc", name="qk0_4")
    nc.vector.tensor_scalar_add(qk_t4[:], ps_qk4[:], bq_sb[:, 4:5])
    qk[0][4] = qk_t4
    new_vt(0)
    o_sb[0] = o_pool.tile([128, NC_CH, HW], MMDT, name="o0", tag="o")

    # ---- attention: flat (b, h, j) stream with one-step S lookahead ----
    steps = [(b, h, j) for b in range(NB) for h in range(NH) for j in range(8)]

    def s_mm(b, h, j):
        st = ps_st_pool.tile([128, HW], F32, tag="st", name=f"st{b}_{h}_{j}")
        po = 64 * (h % 2)
        q_ap = qk[b][h // 2][po : po + 64, :]
        k_ap = qk[b][4 + h // 2][po : po + 64, :]
        for half in range(2):
            nc.tensor.matmul(
                st[:, half * 512 : (half + 1) * 512],
                k_ap[:, j * 128 : (j + 1) * 128],
                q_ap[:, half * 512 : (half + 1) * 512],
                start=True,
                stop=True,
            )
        return st

    def fillers(b, h, j):
        """Demoted background work keyed to step positions."""
        if b == 0:
            if h == 0 and j == 0:
                load_wt_slice(128, 256)      # wt m=1
                vt_group(0, 0)
            elif h == 0 and j == 2:
                load_wt_slice(640, 768)      # wt m=5
                vt_group(0, 1)
            elif h == 0 and j == 4:
                load_wt_slice(256, 384)      # wt m=2
                vt_group(0, 2)
            elif h == 0 and j == 6:
                load_wt_slice(768, 896)      # wt m=6
                vt_group(0, 3)
            elif h == 1 and j == 0:
                load_wt_slice(384, 512)      # wt m=3
                load_wt_slice(896, 1024)     # wt m=7
                x1 = x_pool.tile([128, NC_CH, HW], F32, name="x1", tag="x")
                for c in range(NC_CH):
                    nc.sync.dma_start(x1[:, c, :], x_d[1, c * 128 : (c + 1) * 128, :])
                x_t[1] = x1
                qk_chunk(0, 1, ps_gp_pool, "gp")
            elif h == 1 and j == 4:
                nc.sync.dma_start(
                    wto_sb[:], wto_d.rearrange("(c p) o -> p c o", p=128)
                )
                nc.sync.dma_start(bout_sb[:], bout_d)
                qk_chunk(0, 5, ps_gp_pool, "gp")
            elif h == 2 and j == 0:
                qk_chunk(0, 2, ps_gp_pool, "gp")
            elif h == 2 and j == 4:
                qk_chunk(0, 6, ps_gp_pool, "gp")
            elif h == 3 and j == 0:
                xn[1] = xn_pool.tile([128, NC_CH, HW], MMDT, name="xn1", tag="xn")
                stats2_1 = gn_pool.tile(
                    [128, NC_CH, 2], F32, tag="stats", name="stats1"
                )
                gnps1 = ps_gp_pool.tile([128, 16], F32, tag="gp", name="gnps1")
                # NOT demoted: the tiny Sqrt sits in the in-order ScalarE
                # queue between exps — its inputs must be computed promptly
                # or every later exp head-of-line blocks behind it.
                with tc.high_priority():
                    for c in range(NC_CH):
                        groupnorm_chunk(1, c, x_t[1], xn[1], stats2_1, gnps1,
                                        scalar_sq=False)
            elif h == 4 and j == 0:
                qk_chunk(0, 3, ps_gp_pool, "gp")
            elif h == 4 and j == 4:
                qk_chunk(0, 7, ps_gp_pool, "gp")
            elif h == 5 and j == 0:
                qk_chunk(1, 0, ps_gp_pool, "gp")
            elif h == 5 and j == 4:
                qk_chunk(1, 4, ps_gp_pool, "gp")
            elif h == 6 and j == 0:
                new_vt(1)
                vt_group(1, 0)
            elif h == 7 and j == 0:
                vt_group(1, 1)
            elif h == 7 and j == 4:
                vt_group(1, 2)
        else:
            if h == 0 and j == 0:
                vt_group(1, 3)
            elif h == 0 and j == 4:
                qk_chunk(1, 1, ps_gp_pool, "gp")
            elif h == 1 and j == 0:
                qk_chunk(1, 5, ps_gp_pool, "gp")
            elif h == 1 and j == 4:
                outproj_r(0, 0, ps_gp_pool, "gp")
            elif h == 2 and j == 0:
                qk_chunk(1, 2, ps_gp_pool, "gp")
            elif h == 2 and j == 4:
                outproj_r(0, 1, ps_gp_pool, "gp")
            elif h == 3 and j == 0:
                qk_chunk(1, 6, ps_gp_pool, "gp")
            elif h == 3 and j == 4:
                outproj_r(0, 2, ps_gp_pool, "gp")
            elif h == 4 and j == 0:
                qk_chunk(1, 3, ps_gp_pool, "gp")
            elif h == 4 and j == 4:
                outproj_r(0, 3, ps_gp_pool, "gp")
            elif h == 5 and j == 0:
                qk_chunk(1, 7, ps_gp_pool, "gp")

    def norm_head(b, h, ps_o, fast):
        """Drain PSUM (O rows + sum row in one copy), normalize into o_sb."""
        if os.environ.get("K_BASENORM", "0") == "1":
            norm_head_base(b, h, ps_o)
            return
        po = 64 * (h % 2)
        o65 = ou_pool.tile([65, HW], F32, tag="ou", name=f"o65_{b}_{h}")
        nc.vector.tensor_copy(o65[:], ps_o[0:65, :])
        # DRAM-bounce the raw sum row across 64 partitions (DMA reads
        # partition 64 fine; engine ops there don't), then reciprocal on
        # the broadcast block at base partition 0 — the approx op's only
        # supported placement — then multiply.
        dr = dram.tile([1, HW], F32, name=f"dr{b}_{h}")
        nc.sync.dma_start(dr[:], o65[64:65, :])
        rbs = rb_pool.tile([64, HW], F32, tag="rbs", name=f"rbs{b}_{h}")
        nc.sync.dma_start(rbs[:], dr[:].to_broadcast((64, HW)))
        rb = rb_pool.tile([64, HW], F32, tag="rb", name=f"rbb{b}_{h}")
        nc.vector.reciprocal_approx_fast(rb[:], rbs[:])
        eng = nc.vector if fast else nc.gpsimd
        eng.tensor_tensor(
            o_sb[b][po : po + 64, h // 2, :], o65[0:64, :], rb[:], op=OP.mult
        )

    def norm_head_base(b, h, ps_o):
        """Baseline-style drain + DRAM-bounce normalization."""
        po = 64 * (h % 2)
        o_un = ou_pool.tile([64, HW], F32, tag="ou", name=f"oun_{b}_{h}")
        nc.vector.tensor_copy(o_un[:], ps_o[0:64, :])
        s_row = r_pool.tile([1, HW], F32, tag="row", name=f"s{b}_{h}")
        nc.vector.tensor_copy(s_row[:], ps_o[64:65, :])
        r_row = r_pool.tile([1, HW], F32, tag="row", name=f"r{b}_{h}")
        nc.vector.reciprocal_approx_fast(r_row[:], s_row[:])
        dr = dram.tile([1, HW], F32, name=f"dr{b}_{h}")
        nc.sync.dma_start(dr[:], r_row[:])
        rb = rb_pool.tile([64, HW], F32, tag="rb", name=f"rbb{b}_{h}")
        nc.sync.dma_start(rb[:], dr[:].to_broadcast((64, HW)))
        eng = nc.gpsimd if not (b == 1 and h == 7) else nc.vector
        eng.tensor_tensor(
            o_sb[b][po : po + 64, h // 2, :], o_un[:], rb[:], op=OP.mult
        )

    pend_st = s_mm(0, 0, 0)
    ps_o = None
    for idx, (b, h, j) in enumerate(steps):
        st = pend_st
        p_t = p_pool.tile([128, HW], MMDT, tag="p_t", name=f"p{b}_{h}_{j}")
        nc.scalar.activation(p_t[:], st[:], AF.Exp, scale=0.125)
        with demoted():
            fillers(b, h, j)
        if idx + 1 < len(steps):
            nb_, nh_, nj_ = steps[idx + 1]
            pend_st = s_mm(nb_, nh_, nj_)
        if j == 0:
            if b == 1 and h == 0:
                o_sb[1] = o_pool.tile([128, NC_CH, HW], MMDT, name="o1", tag="o")
            ps_o = ps_o_pool.tile([128, HW], F32, tag="po", name=f"po{b}_{h}")
        for half in range(2):
            nc.tensor.matmul(
                ps_o[0:65, half * 512 : (half + 1) * 512],
                vt[b][:, j, 65 * h : 65 * h + 65],
                p_t[:, half * 512 : (half + 1) * 512],
                start=(j == 0),
                stop=(j == 7),
            )
        if j == 7:
            fast = b == 1 and h >= 6 and os.environ.get("K_FAST", "1") == "1"
            dem = (b == 0 and os.environ.get("K_DEMOTE", "1") == "1"
                   and os.environ.get("K_BASENORM", "0") != "1")
            if dem:
                with demoted():
                    norm_head(b, h, ps_o, fast)
            else:
                norm_head(b, h, ps_o, fast)

    # ---- final out-proj for batch 1: 4 PSUM tiles in parallel ----
    outproj_r(1, 0, ps_st_pool, "st", final=True)
    outproj_r(1, 1, ps_st_pool, "st", final=True)
    outproj_r(1, 2, ps_o_pool, "po", final=True)
    outproj_r(1, 3, ps_gp_pool, "gp", final=True)


_NC_CACHE = None


def _build():
    global _NC_CACHE
    if _NC_CACHE is not None:
        return _NC_CACHE
    import contextlib

    nc = bacc.Bacc("TRN2", target_bir_lowering=False, debug=False)
    with tile.TileContext(nc) as tc:
        with contextlib.ExitStack() as ctx:
            build_program(nc, tc, ctx)
    nc.compile()
    _NC_CACHE = nc
    return nc


def make_in_maps(x, gamma, beta, w_qkv, b_qkv, w_out, b_out):
    x = np.ascontiguousarray(np.asarray(x, dtype=np.float32))
    gamma = np.asarray(gamma, dtype=np.float32)
    beta = np.asarray(beta, dtype=np.float32)
    w_qkv = np.asarray(w_qkv, dtype=np.float32)
    b_qkv = np.asarray(b_qkv, dtype=np.float32)
    w_out = np.asarray(w_out, dtype=np.float32)
    b_out = np.asarray(b_out, dtype=np.float32)

    B, Cc, H, W = x.shape
    assert (B, Cc, H, W) == (16, 512, 32, 32)

    # host-side weight layout transforms (pure layout; no compute moved
    # off-device except the exact fold of the v-bias: softmax rows sum to 1,
    # so attn @ (v + b_v 1^T) = attn @ v + b_v, and W_out @ b_v folds into b_out)
    wt = np.ascontiguousarray(w_qkv.T)                      # [512, 1536]
    wto = np.ascontiguousarray(w_out.T)                     # [512, 512]
    if os.environ.get("K_BF16", "1") == "1":
        import ml_dtypes

        mmnp = ml_dtypes.bfloat16
    else:
        mmnp = np.float32
    wt = wt.astype(mmnp)
    wto = wto.astype(mmnp)
    b_out_eff = b_out + w_out @ b_qkv[2 * C : 3 * C]
    bq = np.ascontiguousarray(b_qkv[: 2 * C].reshape(8, 128).T)   # [128, 8]
    gam = np.ascontiguousarray(gamma.reshape(NC_CH, 128).T)       # [128, 4]
    bet = np.ascontiguousarray(beta.reshape(NC_CH, 128).T)
    bout = np.ascontiguousarray(b_out_eff.reshape(NC_CH, 128).T)

    # per-chunk group masks: chunk c's 128 channels = groups 2c, 2c+1
    gmask_np = np.zeros((128, NC_CH, 2), dtype=np.float32)
    gmaskT_np = np.zeros((2, NC_CH, 128), dtype=np.float32)
    for c in range(NC_CH):
        gmask_np[0:64, c, 0] = 1.0
        gmask_np[64:128, c, 1] = 1.0
        gmaskT_np[0, c, 0:64] = 1.0
        gmaskT_np[1, c, 64:128] = 1.0

    xr = x.reshape(16, 512, 1024)
    in_maps = []
    for core in range(8):
        in_maps.append(
            {
                "x": np.ascontiguousarray(xr[2 * core : 2 * core + 2]),
                "wt": wt,
                "wto": wto,
                "bq": bq,
                "gam": gam,
                "bet": bet,
                "bout": bout,
                "gmask": gmask_np,
                "gmaskT": gmaskT_np,
                "vtones": np.ones((128, NH, NH, 1), dtype=mmnp),
                "onesb": np.ones((1, 64), dtype=mmnp),
            }
        )
    return in_maps


def kernel(x, gamma, beta, w_qkv, b_qkv, w_out, b_out):
    in_maps = make_in_maps(x, gamma, beta, w_qkv, b_qkv, w_out, b_out)
    nc = _build()
    res = bass_utils.run_bass_kernel_spmd(nc, in_maps, core_ids=list(range(8)))
    out = np.concatenate([r["y"] for r in res.results], axis=0)
    return out.reshape(16, 512, 32, 32).astype(np.float32)
